# revision 2
# baseline (speedup 1.0000x reference)
"""MoE MLP (sigmoid router, top-2, relu^2 experts) on 8 Trainium2 cores.

Sharding: expert-parallel with routing-as-sharding. The host computes the
router in numpy f32 (selection verified to match jax.lax.top_k on this
problem's margins) and uses it to DECIDE PLACEMENT: core e receives exactly
the tokens routed to expert e (padded to CAP = max expert count, ~1071),
pre-scaled by sqrt(combine_weight) so that relu(s*x @ w1)^2 = cw * relu(x @
w1)^2, plus expert e's w1/w2 slices in bf16. Each core computes
y = relu(xg @ w1_e)^2 @ w2_e on device; the host scatter-adds the two expert
contributions per token (the unshard-reduce). Only 2 of 8 experts are live
per token, so this does 1/4 of the dense FLOPs: 4.4 GFLOP/core vs 17.2.

Per-core device layout (all activations transposed; no on-device transposes):
  xgT  [D=1024 on partitions (8 tiles of 128), CAP free]  bf16
  h^T[wc] = w1_e[dc,wc-slice] (native layout = lhsT) @ xgT  -> PSUM f32
  a^T[wc] = relu(h)^2                                       -> SBUF bf16
  y^T[dc] = w2_e[wc,dc-slice] (native layout = lhsT) @ a^T  -> PSUM f32
  staged to SBUF f32, DMA to yT [D, CAP]; host transposes + scatter-adds.

Schedule notes (from perfetto traces):
  - bf16 matmuls run 1 PE cycle/row at any moving size; fp32r needs N>=256.
  - DMA descriptor issue costs ~600ns serialized per engine queue; inputs
    are spread over sync/gpsimd and the up0-critical w1/xT0 issue first.
  - chunk 0 accumulates dc-outer across all 8 PSUM banks so the PE consumes
    (w1,xT) tile pairs as they land; later chunks ping-pong bank pairs to
    hide the per-group start penalty.
  - dummy matmuls on a memset tile warm the PE clock out of its low pstate
    while the first DMAs are in flight.
  - the final (smallest) chunk evacuates through a dedicated 8-buffer stage
    pool and alternates engines/queues so the drain is short parallel chains.
"""

import numpy as np
import ml_dtypes

import concourse.bacc as bacc
import concourse.mybir as mybir
import concourse.tile as tile
from concourse.bass_utils import run_bass_kernel_spmd

N_CORES = 8
B, S, D = 2, 2048, 1024
T = B * S  # 4096
E = 8
W = 1024  # width per expert
NDC = D // 128  # 8 D-chunks
NWC = W // 128  # 8 W-chunks
TOP_K = 2

F32 = mybir.dt.float32
BF16 = mybir.dt.bfloat16
BF16_NP = ml_dtypes.bfloat16
AF = mybir.ActivationFunctionType


def _chunks(cap):
    """Split CAP columns into PSUM-bank-sized (<=512 f32) pieces."""
    out = []
    off = 0
    while off < cap:
        c = min(512, cap - off)
        out.append((off, c))
        off += c
    return out


def build_nc(cap):
    nc = bacc.Bacc(
        "TRN2", target_bir_lowering=False, debug=False, num_devices=N_CORES
    )
    xgT = nc.dram_tensor("xgT", [D, cap], BF16, kind="ExternalInput")
    w1 = nc.dram_tensor("w1", [D, W], BF16, kind="ExternalInput")
    w2 = nc.dram_tensor("w2", [W, D], BF16, kind="ExternalInput")
    yT = nc.dram_tensor("yT", [D, cap], F32, kind="ExternalOutput")

    CH = _chunks(cap)
    n_ch = len(CH)

    with tile.TileContext(nc) as tc:
        with (
            tc.tile_pool(name="persist", bufs=1) as persist,
            tc.tile_pool(name="relp", bufs=3) as relp,
            tc.tile_pool(name="ystg", bufs=4) as ystg,
            tc.tile_pool(name="tailstg", bufs=8) as tailstg,
            tc.tile_pool(name="psA", bufs=4, space="PSUM") as psA,
            tc.tile_pool(name="psY", bufs=4, space="PSUM") as psY,
        ):
            # -- PE clock warmup: the PE pstate ramps over ~3us of continuous
            # execution; dummy matmuls on a memset tile start the ramp while
            # the first real DMAs are still in flight.
            wsrc = persist.tile([128, 512], BF16, tag="wsrc", name="wsrc")
            nc.gpsimd.memset(wsrc[:], 0)
            warm = psY.tile([128, 512], F32, tag="y", name="warm")
            for i in range(4):
                nc.tensor.matmul(
                    warm[:],
                    wsrc[:, 0:128],
                    wsrc[:],
                    start=(i == 0),
                    stop=(i == 3),
                )

            # -- input DMAs, issue spread over 4 engine queues (DIRECT2D issue
            # serializes at ~600ns per descriptor on a single queue)
            w1s = []
            xTc = {}
            for dc in range(NDC):
                t = persist.tile([128, W], BF16, tag=f"w1_{dc}", name=f"w1_{dc}")
                nc.sync.dma_start(t[:], w1[dc * 128 : (dc + 1) * 128, :])
                w1s.append(t)
                off, c = CH[0]
                tx = persist.tile([128, c], BF16, tag=f"xT_{dc}_0", name=f"xT_{dc}_0")
                nc.gpsimd.dma_start(
                    tx[:], xgT[dc * 128 : (dc + 1) * 128, off : off + c]
                )
                xTc[(dc, 0)] = tx
            # non-critical inputs issue AFTER the up0-critical w1/xT0 so
            # their transfers don't contend for DMA rings in the fill window
            w2s = []
            for wc in range(NWC):
                t = persist.tile([128, D], BF16, tag=f"w2_{wc}", name=f"w2_{wc}")
                eng = nc.sync if wc < 4 else nc.gpsimd
                eng.dma_start(t[:], w2[wc * 128 : (wc + 1) * 128, :])
                w2s.append(t)
            for ci in range(1, n_ch):
                off, c = CH[ci]
                eng = nc.gpsimd if ci % 2 == 1 else nc.sync
                for dc in range(NDC):
                    tx = persist.tile([128, c], BF16, tag=f"xT_{dc}_{ci}", name=f"xT_{dc}_{ci}")
                    eng.dma_start(
                        tx[:], xgT[dc * 128 : (dc + 1) * 128, off : off + c]
                    )
                    xTc[(dc, ci)] = tx

            aTc = {
                (wc, ci): persist.tile([128, CH[ci][1]], BF16, tag=f"aT_{wc}_{ci}", name=f"aT_{wc}_{ci}")
                for wc in range(NWC)
                for ci in range(n_ch)
            }

            def act_chain(ci, wc, h, c, off):
                rel = relp.tile([128, 512], F32, tag="rel", name=f"rel_{ci}_{wc}")
                nc.scalar.activation(rel[:, :c], h[:, :c], AF.Relu)
                a = aTc[(wc, ci)]
                nc.vector.tensor_mul(a[:], rel[:, :c], rel[:, :c])

            def up_chunk0():
                # dc-outer over ALL 8 PSUM banks: the PE consumes (w1,xT)
                # tile-pairs as the DMAs land instead of waiting for all 8.
                off, c = CH[0]
                hs = [
                    (psA if wc < 4 else psY).tile(
                        [128, 512], F32, tag=("h" if wc < 4 else "y"), name=f"h0_{wc}"
                    )
                    for wc in range(NWC)
                ]
                for dc in range(NDC):
                    for wc in range(NWC):
                        nc.tensor.matmul(
                            hs[wc][:, :c],
                            w1s[dc][:, wc * 128 : (wc + 1) * 128],
                            xTc[(dc, 0)][:],
                            start=(dc == 0),
                            stop=(dc == NDC - 1),
                        )
                for wc in range(NWC):
                    act_chain(0, wc, hs[wc], c, off)

            def up_chunk(ci):
                # ping-pong pairs of accumulation groups: interleaving two
                # PSUM banks hides the per-group start/stop pipeline penalty
                off, c = CH[ci]
                for wcp in range(0, NWC, 2):
                    ha = psA.tile([128, 512], F32, tag="h", name=f"h_{ci}_{wcp}")
                    hb = psA.tile([128, 512], F32, tag="h", name=f"h_{ci}_{wcp + 1}")
                    for dc in range(NDC):
                        for h, wc in ((ha, wcp), (hb, wcp + 1)):
                            nc.tensor.matmul(
                                h[:, :c],
                                w1s[dc][:, wc * 128 : (wc + 1) * 128],
                                xTc[(dc, ci)][:],
                                start=(dc == 0),
                                stop=(dc == NDC - 1),
                            )
                    act_chain(ci, wcp, ha, c, off)
                    act_chain(ci, wcp + 1, hb, c, off)

            def evac(ci, dc, y, c, off):
                last = ci == n_ch - 1
                pool = tailstg if last else ystg
                st = pool.tile([128, 512], F32, tag="yst", name=f"yst_{ci}_{dc}")
                # per-chunk dedicated engine queues so one chunk's trailing
                # evacuations never sit in front of the next chunk's
                use_scalar = (dc % 2 == 0) if last else (ci % 2 == 0)
                if use_scalar:
                    nc.scalar.activation(st[:, :c], y[:, :c], AF.Copy)
                    nc.sync.dma_start(
                        yT[dc * 128 : (dc + 1) * 128, off : off + c], st[:, :c]
                    )
                else:
                    nc.vector.tensor_copy(st[:, :c], y[:, :c])
                    nc.gpsimd.dma_start(
                        yT[dc * 128 : (dc + 1) * 128, off : off + c], st[:, :c]
                    )

            def down_chunk(ci):
                off, c = CH[ci]
                for dcp in range(0, NDC, 2):
                    if ci == n_ch - 1 and (dcp // 2) % 2 == 0:
                        pool, ptag = psA, "h"
                    else:
                        pool, ptag = psY, "y"
                    ya = pool.tile([128, 512], F32, tag=ptag, name=f"y_{ci}_{dcp}")
                    yb = pool.tile([128, 512], F32, tag=ptag, name=f"y_{ci}_{dcp + 1}")
                    for wc in range(NWC):
                        for y, dc in ((ya, dcp), (yb, dcp + 1)):
                            nc.tensor.matmul(
                                y[:, :c],
                                w2s[wc][:, dc * 128 : (dc + 1) * 128],
                                aTc[(wc, ci)][:],
                                start=(wc == 0),
                                stop=(wc == NWC - 1),
                            )
                    evac(ci, dcp, ya, c, off)
                    evac(ci, dcp + 1, yb, c, off)

            # software pipeline: PE works on up(ci+1) while DVE finishes a(ci)
            up_chunk0()
            for ci in range(1, n_ch):
                up_chunk(ci)
                down_chunk(ci - 1)
            down_chunk(n_ch - 1)

    nc.compile()
    return nc


_NC_CACHE = {}


def get_nc(cap):
    if cap not in _NC_CACHE:
        _NC_CACHE[cap] = build_nc(cap)
    return _NC_CACHE[cap]


def route(x, router_w):
    """Host router: numpy f32, same math as the reference. Returns
    per-expert token index lists and normalized top-2 combine weights."""
    xf = np.ascontiguousarray(np.asarray(x, dtype=np.float32).reshape(T, D))
    rw = np.asarray(router_w, dtype=np.float32)
    logits = xf @ rw.T  # [T, E]
    probs = (1.0 / (1.0 + np.exp(-logits))).astype(np.float32)
    sel = np.argsort(-probs, axis=1)[:, :TOP_K]  # [T, K]
    top_w = np.take_along_axis(probs, sel, axis=1)
    top_w = top_w / (top_w.sum(axis=1, keepdims=True) + np.float32(1e-20))
    idx = [np.nonzero((sel == e).any(axis=1))[0] for e in range(E)]
    cw_tok = np.zeros((T, E), dtype=np.float32)
    np.put_along_axis(cw_tok, sel, top_w.astype(np.float32), axis=1)
    return xf, idx, cw_tok


def prepare(x, router_w, w1, w2):
    xf, idx, cw_tok = route(x, router_w)
    w1 = np.asarray(w1, dtype=np.float32)
    w2 = np.asarray(w2, dtype=np.float32)
    cap = max(len(i) for i in idx)
    in_maps = []
    for e in range(E):
        cnt = len(idx[e])
        xg = np.zeros((cap, D), dtype=np.float32)
        # fold sqrt(cw) into the tokens: relu(s*x @ w1)^2 = cw * relu(x @ w1)^2
        xg[:cnt] = xf[idx[e]] * np.sqrt(cw_tok[idx[e], e])[:, None]
        in_maps.append(
            {
                "xgT": np.ascontiguousarray(xg.T).astype(BF16_NP),
                "w1": np.ascontiguousarray(
                    w1[:, e * W : (e + 1) * W]
                ).astype(BF16_NP),
                "w2": np.ascontiguousarray(
                    w2[e * W : (e + 1) * W, :]
                ).astype(BF16_NP),
            }
        )
    return cap, in_maps, idx


def combine(res, idx):
    out = np.zeros((T, D), dtype=np.float32)
    for e in range(E):
        cnt = len(idx[e])
        yT = np.asarray(res.results[e]["yT"], dtype=np.float32)  # [D, cap]
        out[idx[e]] += yT.T[:cnt]
    return out.reshape(B, S, D)


def run(x, router_w, w1, w2, trace=False):
    cap, in_maps, idx = prepare(x, router_w, w1, w2)
    nc = get_nc(cap)
    res = run_bass_kernel_spmd(nc, in_maps, list(range(N_CORES)), trace=trace)
    return combine(res, idx), res


def kernel(x, router_w, w1, w2):
    out, _ = run(x, router_w, w1, w2)
    return out.astype(np.float32)


# revision 3
# speedup vs baseline: 1.1786x; 1.1786x over previous
"""MoE MLP (sigmoid router, top-2, relu^2 experts) on 8 Trainium2 cores.

Sharding: expert-parallel with routing-as-sharding. The host computes the
router in numpy f32 (selection verified to match jax.lax.top_k on this
problem's margins) and uses it to DECIDE PLACEMENT: core e receives exactly
the tokens routed to expert e (padded to CAP = max expert count, ~1071),
pre-scaled by sqrt(combine_weight) so that relu(s*x @ w1)^2 = cw * relu(x @
w1)^2, plus expert e's w1/w2 slices in bf16. Each core computes
y = relu(xg @ w1_e)^2 @ w2_e on device; the host scatter-adds the two expert
contributions per token (the unshard-reduce). Only 2 of 8 experts are live
per token, so this does 1/4 of the dense FLOPs: 4.4 GFLOP/core vs 17.2.

Per-core device layout (all activations transposed; no on-device transposes):
  xgT  [D=1024 on partitions (8 tiles of 128), CAP free]  bf16
  h^T[wc] = w1_e[dc,wc-slice] (native layout = lhsT) @ xgT  -> PSUM f32
  a^T[wc] = relu(h)^2                                       -> SBUF bf16
  y^T[dc] = w2_e[wc,dc-slice] (native layout = lhsT) @ a^T  -> PSUM f32
  staged to SBUF f32 and DMA'd out; host transposes + scatter-adds.

Schedule notes (from perfetto traces):
  - bf16 matmuls run 1 PE cycle/row at any moving size; fp32r needs N>=256.
  - DMA descriptor issue costs ~600ns serialized per engine queue; inputs
    spread over sync/gpsimd with the up0-critical w1/xT0 issued first.
  - chunk 0 accumulates dc-outer across all 8 PSUM banks so the PE consumes
    (w1,xT) tile pairs as they land; later chunks ping-pong bank pairs to
    hide the per-group start penalty.
  - dummy matmuls on a memset tile warm the PE clock out of its low pstate
    while the first DMAs are in flight.
  - the final partial chunk evacuates via parallel scalar/vector copies into
    one partition-major 3D stage and leaves as a SINGLE DMA (yT2) — the
    drain chain after the last matmul is just copies + one issue + barrier.
"""

import numpy as np
import ml_dtypes

import concourse.bacc as bacc
import concourse.mybir as mybir
import concourse.tile as tile
from concourse.bass_utils import run_bass_kernel_spmd

N_CORES = 8
B, S, D = 2, 2048, 1024
T = B * S  # 4096
E = 8
W = 1024  # width per expert
NDC = D // 128  # 8 D-chunks
NWC = W // 128  # 8 W-chunks
TOP_K = 2

F32 = mybir.dt.float32
BF16 = mybir.dt.bfloat16
BF16_NP = ml_dtypes.bfloat16
AF = mybir.ActivationFunctionType


def _chunks(cap):
    """Split CAP columns into PSUM-bank-sized (<=512 f32) pieces."""
    out = []
    off = 0
    while off < cap:
        c = min(512, cap - off)
        out.append((off, c))
        off += c
    return out


def build_nc(cap):
    nc = bacc.Bacc(
        "TRN2", target_bir_lowering=False, debug=False, num_devices=N_CORES
    )
    xgT = nc.dram_tensor("xgT", [D, cap], BF16, kind="ExternalInput")
    w1 = nc.dram_tensor("w1", [D, W], BF16, kind="ExternalInput")
    w2 = nc.dram_tensor("w2", [W, D], BF16, kind="ExternalInput")
    yT = nc.dram_tensor("yT", [D, cap], F32, kind="ExternalOutput")

    CH = _chunks(cap)
    n_ch = len(CH)
    # final partial chunk exits via ONE DMA from a partition-major 3D stage;
    # [p, dc, col] iteration order matches on both sides
    rem = CH[-1][1] if CH[-1][1] < 512 else 0
    yT2 = (
        nc.dram_tensor("yT2", [128, NDC, rem], F32, kind="ExternalOutput")
        if rem
        else None
    )

    with tile.TileContext(nc) as tc:
        with (
            tc.tile_pool(name="persist", bufs=1) as persist,
            tc.tile_pool(name="relp", bufs=3) as relp,
            tc.tile_pool(name="ystg", bufs=4) as ystg,
            tc.tile_pool(name="tailstg", bufs=1) as tailstg,
            tc.tile_pool(name="psA", bufs=4, space="PSUM") as psA,
            tc.tile_pool(name="psY", bufs=4, space="PSUM") as psY,
        ):
            # -- PE clock warmup: the PE pstate ramps over ~3us of continuous
            # execution; dummy matmuls on a memset tile start the ramp while
            # the first real DMAs are still in flight.
            wsrc = persist.tile([128, 512], BF16, tag="wsrc", name="wsrc")
            nc.gpsimd.memset(wsrc[:], 0)
            warm = psY.tile([128, 512], F32, tag="y", name="warm")
            for i in range(4):
                nc.tensor.matmul(
                    warm[:],
                    wsrc[:, 0:128],
                    wsrc[:],
                    start=(i == 0),
                    stop=(i == 3),
                )

            # -- input DMAs, issue spread over 4 engine queues (DIRECT2D issue
            # serializes at ~600ns per descriptor on a single queue)
            w1s = []
            xTc = {}
            for dc in range(NDC):
                t = persist.tile([128, W], BF16, tag=f"w1_{dc}", name=f"w1_{dc}")
                nc.sync.dma_start(t[:], w1[dc * 128 : (dc + 1) * 128, :])
                w1s.append(t)
                off, c = CH[0]
                tx = persist.tile([128, c], BF16, tag=f"xT_{dc}_0", name=f"xT_{dc}_0")
                nc.gpsimd.dma_start(
                    tx[:], xgT[dc * 128 : (dc + 1) * 128, off : off + c]
                )
                xTc[(dc, 0)] = tx
            # non-critical inputs issue AFTER the up0-critical w1/xT0 so
            # their transfers don't contend for DMA rings in the fill window
            w2s = []
            for wc in range(NWC):
                t = persist.tile([128, D], BF16, tag=f"w2_{wc}", name=f"w2_{wc}")
                eng = nc.sync if wc < 4 else nc.gpsimd
                eng.dma_start(t[:], w2[wc * 128 : (wc + 1) * 128, :])
                w2s.append(t)
            for ci in range(1, n_ch):
                off, c = CH[ci]
                eng = nc.gpsimd if ci % 2 == 1 else nc.sync
                for dc in range(NDC):
                    tx = persist.tile([128, c], BF16, tag=f"xT_{dc}_{ci}", name=f"xT_{dc}_{ci}")
                    eng.dma_start(
                        tx[:], xgT[dc * 128 : (dc + 1) * 128, off : off + c]
                    )
                    xTc[(dc, ci)] = tx

            aTc = {
                (wc, ci): persist.tile([128, CH[ci][1]], BF16, tag=f"aT_{wc}_{ci}", name=f"aT_{wc}_{ci}")
                for wc in range(NWC)
                for ci in range(n_ch)
            }

            def act_chain(ci, wc, h, c, off):
                rel = relp.tile([128, 512], F32, tag="rel", name=f"rel_{ci}_{wc}")
                nc.scalar.activation(rel[:, :c], h[:, :c], AF.Relu)
                a = aTc[(wc, ci)]
                nc.vector.tensor_mul(a[:], rel[:, :c], rel[:, :c])

            def up_chunk0():
                # dc-outer over ALL 8 PSUM banks: the PE consumes (w1,xT)
                # tile-pairs as the DMAs land instead of waiting for all 8.
                off, c = CH[0]
                hs = [
                    (psA if wc < 4 else psY).tile(
                        [128, 512], F32, tag=("h" if wc < 4 else "y"), name=f"h0_{wc}"
                    )
                    for wc in range(NWC)
                ]
                for dc in range(NDC):
                    for wc in range(NWC):
                        nc.tensor.matmul(
                            hs[wc][:, :c],
                            w1s[dc][:, wc * 128 : (wc + 1) * 128],
                            xTc[(dc, 0)][:],
                            start=(dc == 0),
                            stop=(dc == NDC - 1),
                        )
                for wc in range(NWC):
                    act_chain(0, wc, hs[wc], c, off)

            def up_chunk(ci):
                # ping-pong pairs of accumulation groups: interleaving two
                # PSUM banks hides the per-group start/stop pipeline penalty
                off, c = CH[ci]
                for wcp in range(0, NWC, 2):
                    ha = psA.tile([128, 512], F32, tag="h", name=f"h_{ci}_{wcp}")
                    hb = psA.tile([128, 512], F32, tag="h", name=f"h_{ci}_{wcp + 1}")
                    for dc in range(NDC):
                        for h, wc in ((ha, wcp), (hb, wcp + 1)):
                            nc.tensor.matmul(
                                h[:, :c],
                                w1s[dc][:, wc * 128 : (wc + 1) * 128],
                                xTc[(dc, ci)][:],
                                start=(dc == 0),
                                stop=(dc == NDC - 1),
                            )
                    act_chain(ci, wcp, ha, c, off)
                    act_chain(ci, wcp + 1, hb, c, off)

            st3 = (
                tailstg.tile([128, NDC, rem], F32, tag="st3", name="st3")
                if rem
                else None
            )

            def evac(ci, dc, y, c, off):
                if rem and ci == n_ch - 1:
                    # copies only (parallel on scalar/vector); single DMA at
                    # the end of the chunk drains the whole tail at once
                    if dc % 2 == 0:
                        nc.scalar.activation(st3[:, dc, :], y[:, :c], AF.Copy)
                    else:
                        nc.vector.tensor_copy(st3[:, dc, :], y[:, :c])
                    return
                st = ystg.tile([128, 512], F32, tag="yst", name=f"yst_{ci}_{dc}")
                # per-chunk dedicated engine queues so one chunk's trailing
                # evacuations never sit in front of the next chunk's
                if ci % 2 == 0:
                    nc.scalar.activation(st[:, :c], y[:, :c], AF.Copy)
                    nc.sync.dma_start(
                        yT[dc * 128 : (dc + 1) * 128, off : off + c], st[:, :c]
                    )
                else:
                    nc.vector.tensor_copy(st[:, :c], y[:, :c])
                    nc.gpsimd.dma_start(
                        yT[dc * 128 : (dc + 1) * 128, off : off + c], st[:, :c]
                    )

            def down_chunk(ci):
                off, c = CH[ci]
                for dcp in range(0, NDC, 2):
                    if ci == n_ch - 1 and (dcp // 2) % 2 == 0:
                        pool, ptag = psA, "h"
                    else:
                        pool, ptag = psY, "y"
                    ya = pool.tile([128, 512], F32, tag=ptag, name=f"y_{ci}_{dcp}")
                    yb = pool.tile([128, 512], F32, tag=ptag, name=f"y_{ci}_{dcp + 1}")
                    for wc in range(NWC):
                        for y, dc in ((ya, dcp), (yb, dcp + 1)):
                            nc.tensor.matmul(
                                y[:, :c],
                                w2s[wc][:, dc * 128 : (dc + 1) * 128],
                                aTc[(wc, ci)][:],
                                start=(wc == 0),
                                stop=(wc == NWC - 1),
                            )
                    evac(ci, dcp, ya, c, off)
                    evac(ci, dcp + 1, yb, c, off)

            # software pipeline: PE works on up(ci+1) while DVE finishes a(ci)
            up_chunk0()
            for ci in range(1, n_ch):
                up_chunk(ci)
                down_chunk(ci - 1)
            down_chunk(n_ch - 1)
            if rem:
                nc.sync.dma_start(yT2[:], st3[:])

    nc.compile()
    return nc


_NC_CACHE = {}


def get_nc(cap):
    if cap not in _NC_CACHE:
        _NC_CACHE[cap] = build_nc(cap)
    return _NC_CACHE[cap]


def route(x, router_w):
    """Host router: numpy f32, same math as the reference. Returns
    per-expert token index lists and normalized top-2 combine weights."""
    xf = np.ascontiguousarray(np.asarray(x, dtype=np.float32).reshape(T, D))
    rw = np.asarray(router_w, dtype=np.float32)
    logits = xf @ rw.T  # [T, E]
    probs = (1.0 / (1.0 + np.exp(-logits))).astype(np.float32)
    sel = np.argsort(-probs, axis=1)[:, :TOP_K]  # [T, K]
    top_w = np.take_along_axis(probs, sel, axis=1)
    top_w = top_w / (top_w.sum(axis=1, keepdims=True) + np.float32(1e-20))
    idx = [np.nonzero((sel == e).any(axis=1))[0] for e in range(E)]
    cw_tok = np.zeros((T, E), dtype=np.float32)
    np.put_along_axis(cw_tok, sel, top_w.astype(np.float32), axis=1)
    return xf, idx, cw_tok


def prepare(x, router_w, w1, w2):
    xf, idx, cw_tok = route(x, router_w)
    w1 = np.asarray(w1, dtype=np.float32)
    w2 = np.asarray(w2, dtype=np.float32)
    cap = max(len(i) for i in idx)
    in_maps = []
    for e in range(E):
        cnt = len(idx[e])
        xg = np.zeros((cap, D), dtype=np.float32)
        # fold sqrt(cw) into the tokens: relu(s*x @ w1)^2 = cw * relu(x @ w1)^2
        xg[:cnt] = xf[idx[e]] * np.sqrt(cw_tok[idx[e], e])[:, None]
        in_maps.append(
            {
                "xgT": np.ascontiguousarray(xg.T).astype(BF16_NP),
                "w1": np.ascontiguousarray(
                    w1[:, e * W : (e + 1) * W]
                ).astype(BF16_NP),
                "w2": np.ascontiguousarray(
                    w2[e * W : (e + 1) * W, :]
                ).astype(BF16_NP),
            }
        )
    return cap, in_maps, idx


def combine(res, idx, cap):
    out = np.zeros((T, D), dtype=np.float32)
    for e in range(E):
        cnt = len(idx[e])
        yT = np.asarray(res.results[e]["yT"], dtype=np.float32)  # [D, cap]
        if "yT2" in res.results[e]:
            y2 = np.asarray(res.results[e]["yT2"], dtype=np.float32)
            rem = y2.shape[2]
            yT = yT.copy()
            yT[:, cap - rem :] = y2.transpose(1, 0, 2).reshape(D, rem)
        out[idx[e]] += yT.T[:cnt]
    return out.reshape(B, S, D)


def run(x, router_w, w1, w2, trace=False):
    cap, in_maps, idx = prepare(x, router_w, w1, w2)
    nc = get_nc(cap)
    res = run_bass_kernel_spmd(nc, in_maps, list(range(N_CORES)), trace=trace)
    return combine(res, idx, cap), res


def kernel(x, router_w, w1, w2):
    out, _ = run(x, router_w, w1, w2)
    return out.astype(np.float32)
